# revision 23
# baseline (speedup 1.0000x reference)
"""CMADE ensemble kernel for 8 TRN2 NeuronCores.

Problem: B=16 binary-masked 4-layer MLPs (96 -> 1024 -> 1024 -> 1024 -> 64)
over the same N=4096 batch; output = mean over the 16 masks.

Strategy: data-parallel over the batch N — each core takes 512 rows and runs
all 16 masked MLPs on them, accumulating the final-layer outputs of all 16
masks into a single PSUM tile. The ensemble mean is finished locally
(scale 1/16 + bias), so no inter-core collective is needed; the host
concatenates the 8 row-slices. Masked weights (W.T * M, bf16) are produced
on-chip by the vector engine from resident W.T tiles and streamed mask
tiles, double-buffered across masks. Matmuls run in bf16 with fp32 PSUM
accumulation; activations stay transposed ([feature, batch]) so weights are
the stationary operand.
"""

import numpy as np
import ml_dtypes

from concourse import bacc
import concourse.bass as bass
import concourse.mybir as mybir
import concourse.tile as tile
from concourse.bass_utils import run_bass_kernel_spmd
from concourse.masks import make_identity

BF16 = ml_dtypes.bfloat16

N = 4096
B = 16
NCORES = 8
NLOC = N // NCORES           # 512 batch rows per core
D_IN = 96
H = 1024
D_OUT = 64
KT = H // 128                # 8 k-tiles for the 1024-wide dims

# set True (or env BASS_TRACE=1) before calling kernel() to capture an NTFF
# profile; exec time lands in LAST_RESULT.exec_time_ns
TRACE = False
LAST_RESULT = None

_CACHE = {}


def _ensure_ntff_hook():
    """The agent image's antenv lacks axon_hooks; reconstruct the NTFF
    profile hook from trn_agent_boot so trace=True yields exec_time_ns."""
    import sys as _sys
    import types
    try:
        from antenv import axon_hooks  # noqa: F401
        return
    except ImportError:
        pass
    import antenv
    import concourse.bass_utils as _bu
    _bu.upload_artifacts = lambda tmpdir: tmpdir  # zero-egress container
    holder = {}
    mod = types.ModuleType("antenv.axon_hooks")
    mod.set_axon_ntff_profile_hook = lambda h: holder.__setitem__("h", h)
    mod.get_axon_ntff_profile_hook = lambda: holder.get("h")
    _sys.modules["antenv.axon_hooks"] = mod
    antenv.axon_hooks = mod
    from trn_agent_boot.trn_boot import _ntff_profile_via_ctypes
    mod.set_axon_ntff_profile_hook(
        _ntff_profile_via_ctypes("/opt/axon/libaxon_pjrt.so"))


def _build_graph():
    f32 = mybir.dt.float32
    bf = mybir.dt.bfloat16
    nc = bacc.Bacc("TRN2", target_bir_lowering=False, debug=False,
                   num_devices=NCORES)

    # ---- I/O ----
    xyT_d = nc.dram_tensor("xyT", [D_IN, NLOC], bf, kind="ExternalInput")
    wt0_d = nc.dram_tensor("wt0", [D_IN, H], bf, kind="ExternalInput")
    wt1_d = nc.dram_tensor("wt1", [128, KT * H], bf, kind="ExternalInput")
    wt2_d = nc.dram_tensor("wt2", [128, KT * H], bf, kind="ExternalInput")
    wt3_d = nc.dram_tensor("wt3", [128, KT * D_OUT], bf, kind="ExternalInput")
    m0_d = nc.dram_tensor("m0", [B, D_IN, H], bf, kind="ExternalInput")
    m1_d = nc.dram_tensor("m1", [B, 128, KT * H], bf, kind="ExternalInput")
    m2_d = nc.dram_tensor("m2", [B, 128, KT * H], bf, kind="ExternalInput")
    m3_d = nc.dram_tensor("m3", [B, 128, KT * D_OUT], bf, kind="ExternalInput")
    b0_d = nc.dram_tensor("b0r", [128, KT], f32, kind="ExternalInput")
    b1_d = nc.dram_tensor("b1r", [128, KT], f32, kind="ExternalInput")
    b2_d = nc.dram_tensor("b2r", [128, KT], f32, kind="ExternalInput")
    b3_d = nc.dram_tensor("b3r", [D_OUT, 1], f32, kind="ExternalInput")
    out_d = nc.dram_tensor("out", [NLOC, D_OUT], f32, kind="ExternalOutput")

    relu = mybir.ActivationFunctionType.Relu
    iden = mybir.ActivationFunctionType.Identity

    from contextlib import ExitStack
    with tile.TileContext(nc) as tc, ExitStack() as ctx:
        const = ctx.enter_context(tc.tile_pool(name="const", bufs=1))
        mstg = ctx.enter_context(tc.tile_pool(name="mstg", bufs=3))
        apool = ctx.enter_context(tc.tile_pool(name="act", bufs=2))
        # all 8 banks for working psums (incl. layer-3 and final transposes)
        pspool = ctx.enter_context(tc.tile_pool(name="ps", bufs=8, space="PSUM"))
        finp = ctx.enter_context(tc.tile_pool(name="fin", bufs=4))
        outtp = ctx.enter_context(tc.tile_pool(name="outT", bufs=4))

        # ---- resident constants; mask-0 critical path interleaved ----
        xyT = const.tile([D_IN, NLOC], bf, tag="xyT")
        nc.sync.dma_start(xyT[:], xyT_d[:])
        wt0 = const.tile([D_IN, H], bf, tag="wt0")
        nc.sync.dma_start(wt0[:], wt0_d[:])
        b0t = const.tile([128, KT], f32, tag="b0t")
        nc.sync.dma_start(b0t[:], b0_d[:])
        b1t = const.tile([128, KT], f32, tag="b1t")
        nc.sync.dma_start(b1t[:], b1_d[:])
        b2t = const.tile([128, KT], f32, tag="b2t")
        nc.sync.dma_start(b2t[:], b2_d[:])
        b3t = const.tile([D_OUT, 1], f32, tag="b3t")
        nc.sync.dma_start(b3t[:], b3_d[:])
        wt1 = const.tile([128, KT * H], bf, tag="wt1")
        wt2 = const.tile([128, KT * H], bf, tag="wt2")
        wt3 = const.tile([128, KT * D_OUT], bf, tag="wt3")

        # masked-weight producer for mask b: DMA the mask tile, multiply the
        # resident W.T slice into it IN PLACE (bf16, vector engine). m1 tiles
        # stream on gpsimd, m2 on sync; deep bufs give ~1.5 masks lookahead.
        # For b == 0 the W.T k-slices are interleaved so layer 1 starts ASAP.
        def make_mw(b):
            mw0 = mstg.tile([D_IN, H], bf, tag="m0s", bufs=3,
                            name=f"mw0_{b}")
            nc.gpsimd.dma_start(mw0[:], m0_d[b])
            nc.vector.tensor_mul(mw0[:], wt0[:], mw0[:])
            mw1, mw2 = [], []
            for k in range(KT):
                sl = slice(k * H, (k + 1) * H)
                if b == 0:
                    nc.sync.dma_start(wt1[:, sl], wt1_d[:, sl])
                mt = mstg.tile([128, H], bf, tag="m1s", bufs=18,
                               name=f"mw1_{b}_{k}")
                nc.gpsimd.dma_start(mt[:], m1_d[b][:, sl])
                nc.vector.tensor_mul(mt[:], wt1[:, sl], mt[:])
                mw1.append(mt)
            for k in range(KT):
                sl = slice(k * H, (k + 1) * H)
                if b == 0:
                    nc.sync.dma_start(wt2[:, sl], wt2_d[:, sl])
                mt = mstg.tile([128, H], bf, tag="m2s", bufs=18,
                               name=f"mw2_{b}_{k}")
                nc.sync.dma_start(mt[:], m2_d[b][:, sl])
                nc.vector.tensor_mul(mt[:], wt2[:, sl], mt[:])
                mw2.append(mt)
            if b == 0:
                nc.sync.dma_start(wt3[:], wt3_d[:])
            mw3 = mstg.tile([128, KT * D_OUT], bf, tag="m3s", bufs=3,
                            name=f"mw3_{b}")
            nc.gpsimd.dma_start(mw3[:], m3_d[b])
            nc.vector.tensor_mul(mw3[:], wt3[:], mw3[:])
            return mw0, mw1, mw2, mw3

        # psum -> sbuf relu drain, alternating between the Scalar and Vector
        # engines so the drain latency (which paces psum slot recycling)
        # halves; DVE computes max(psum + bias, 0) via tensor_scalar
        add_op = mybir.AluOpType.add
        max_op = mybir.AluOpType.max

        def drain(at, ps, bt, m):
            nc.scalar.activation(at[:], ps[:], relu, bias=bt[:, m:m + 1])

        # middle layers run k-outer over m-halves of 4 so the PE consumes
        # the previous layer's activations at the rate ACT produces them
        # (m-outer stalls ~4us per mask waiting for all 8 ACTs)
        def mid_layer(b, lname, mw, a_in, bt):
            a_out = [None] * KT
            for half in range(2):
                ms = range(half * 4, half * 4 + 4)
                pss = []
                for m in ms:
                    ps = pspool.tile([128, NLOC], f32, tag="ps",
                                     name=f"ps_{lname}_{b}_{m}")
                    pss.append(ps)
                for k in range(KT):
                    for mi, m in enumerate(ms):
                        nc.tensor.matmul(pss[mi][:],
                                         mw[k][:, m * 128:(m + 1) * 128],
                                         a_in[k][:],
                                         start=(k == 0), stop=(k == KT - 1))
                for mi, m in enumerate(ms):
                    at = apool.tile([128, NLOC], bf, tag=f"{lname}_{m}",
                                    name=f"{lname}_{b}_{m}")
                    drain(at, pss[mi], bt, m)
                    a_out[m] = at
            return a_out

        # ---- main compute: all 16 masks over this core's 512 rows ----
        # ensemble sum accumulates in an fp32 SBUF tile via DVE adds, so no
        # psum bank is pinned for the whole kernel
        acc = const.tile([D_OUT, NLOC], f32, tag="acc")
        for b in range(B):
            mw0, mw1, mw2, mw3 = make_mw(b)
            # layer 0: [96] -> [1024]
            a0 = []
            for m in range(KT):
                ps = pspool.tile([128, NLOC], f32, tag="ps",
                                 name=f"ps_a0_{b}_{m}")
                nc.tensor.matmul(ps[:], mw0[:, m * 128:(m + 1) * 128],
                                 xyT[:], start=True, stop=True)
                at = apool.tile([128, NLOC], bf, tag=f"a0_{m}",
                                name=f"a0_{b}_{m}")
                drain(at, ps, b0t, m)
                a0.append(at)
            a1 = mid_layer(b, "a1", mw1, a0, b1t)
            a2 = mid_layer(b, "a2", mw2, a1, b2t)
            # layer 3: [1024] -> [64]; per-mask psum, summed into acc
            ps3 = pspool.tile([D_OUT, NLOC], f32, tag="ps",
                              name=f"ps3_{b}")
            for k in range(KT):
                lo = k * D_OUT
                nc.tensor.matmul(ps3[:], mw3[:, lo:lo + D_OUT], a2[k][:],
                                 start=(k == 0), stop=(k == KT - 1))
            if b == 0:
                nc.vector.tensor_copy(acc[:], ps3[:])
            else:
                nc.vector.tensor_add(acc[:], acc[:], ps3[:])

        # ---- finalize: ensemble mean + bias, transpose, store ----
        ident = const.tile([128, 128], f32, tag="ident")
        make_identity(nc, ident[:])
        for t in range(NLOC // 128):
            fin = finp.tile([D_OUT, 128], f32, tag="fin", name=f"fin_{t}")
            nc.scalar.activation(fin[:], acc[:, t * 128:(t + 1) * 128], iden,
                                 bias=b3t[:, 0:1], scale=1.0 / B)
            pst = pspool.tile([128, D_OUT], f32, tag="ps", name=f"pst_{t}")
            nc.tensor.transpose(pst[:], fin[:], ident[:D_OUT, :D_OUT])
            ot = outtp.tile([128, D_OUT], f32, tag="ot", name=f"ot_{t}")
            nc.vector.tensor_copy(ot[:], pst[:])
            nc.sync.dma_start(out_d[t * 128:(t + 1) * 128, :], ot[:])

    nc.compile()
    return nc


def _prep_shared(W0, W1, W2, W3, b0, b1, b2, b3,
                 mask0, mask1, mask2, mask3):
    wt0 = np.ascontiguousarray(W0.T).astype(BF16)

    def kfold(wT, out_w):
        # [1024, out] -> [8, 128, out] -> [128, 8*out]
        return np.ascontiguousarray(
            wT.reshape(KT, 128, out_w).transpose(1, 0, 2).reshape(128, KT * out_w)
        ).astype(BF16)

    def mkfold(m, out_w):
        # [B, 1024, out] -> [B, 8, 128, out] -> [B, 128, 8*out]
        return np.ascontiguousarray(
            m.reshape(B, KT, 128, out_w).transpose(0, 2, 1, 3)
            .reshape(B, 128, KT * out_w)).astype(BF16)

    return dict(
        wt0=wt0, wt1=kfold(W1.T, H), wt2=kfold(W2.T, H),
        wt3=kfold(W3.T, D_OUT),
        m0=mask0.astype(BF16),
        m1=mkfold(mask1, H), m2=mkfold(mask2, H), m3=mkfold(mask3, D_OUT),
        b0r=np.ascontiguousarray(b0.reshape(KT, 128).T).astype(np.float32),
        b1r=np.ascontiguousarray(b1.reshape(KT, 128).T).astype(np.float32),
        b2r=np.ascontiguousarray(b2.reshape(KT, 128).T).astype(np.float32),
        b3r=np.ascontiguousarray(b3.reshape(D_OUT, 1)).astype(np.float32),
    )


def kernel(xy, W0, b0, W1, b1, W2, b2, W3, b3,
           mask0, mask1, mask2, mask3):
    global LAST_RESULT
    xy = np.asarray(xy, np.float32)
    args = [np.asarray(a, np.float32) for a in
            (W0, W1, W2, W3, b0, b1, b2, b3)]
    masks = [np.asarray(m, np.float32) for m in (mask0, mask1, mask2, mask3)]

    if "nc" not in _CACHE:
        _CACHE["nc"] = _build_graph()
    nc = _CACHE["nc"]

    shared = _prep_shared(*args, *masks)
    xyT = np.ascontiguousarray(xy.T).astype(BF16)   # [96, 4096]
    in_maps = []
    for core in range(NCORES):
        im = dict(shared)
        im["xyT"] = np.ascontiguousarray(
            xyT[:, core * NLOC:(core + 1) * NLOC])
        in_maps.append(im)

    if TRACE:
        _ensure_ntff_hook()
    res = run_bass_kernel_spmd(
        nc, in_maps, core_ids=list(range(NCORES)),
        trace=TRACE)
    LAST_RESULT = res
    return np.concatenate(
        [np.asarray(res.results[i]["out"], np.float32)
         for i in range(NCORES)], axis=0)


# revision 32
# speedup vs baseline: 1.2294x; 1.2294x over previous
"""CMADE ensemble kernel for 8 TRN2 NeuronCores.

Problem: B=16 binary-masked 4-layer MLPs (96 -> 1024 -> 1024 -> 1024 -> 64)
over the same N=4096 batch; output = mean over the 16 masks.

Strategy: data-parallel over the batch N — each core takes 512 rows and runs
all 16 masked MLPs on them, accumulating the final-layer outputs of all 16
masks into a single PSUM tile. The ensemble mean is finished locally
(scale 1/16 + bias), so no inter-core collective is needed; the host
concatenates the 8 row-slices. Masked weights (W.T * M, bf16) are produced
on-chip by the vector engine from resident W.T tiles and streamed mask
tiles, double-buffered across masks. Matmuls run in bf16 with fp32 PSUM
accumulation; activations stay transposed ([feature, batch]) so weights are
the stationary operand.
"""

import numpy as np
import ml_dtypes

from concourse import bacc
import concourse.bass as bass
import concourse.mybir as mybir
import concourse.tile as tile
from concourse.bass_utils import run_bass_kernel_spmd
from concourse.masks import make_identity

BF16 = ml_dtypes.bfloat16

N = 4096
B = 16
NCORES = 8
NLOC = N // NCORES           # 512 batch rows per core
D_IN = 96
H = 1024
D_OUT = 64
KT = H // 128                # 8 k-tiles for the 1024-wide dims

# set True (or env BASS_TRACE=1) before calling kernel() to capture an NTFF
# profile; exec time lands in LAST_RESULT.exec_time_ns
TRACE = False
LAST_RESULT = None

_CACHE = {}


def _ensure_ntff_hook():
    """The agent image's antenv lacks axon_hooks; reconstruct the NTFF
    profile hook from trn_agent_boot so trace=True yields exec_time_ns."""
    import sys as _sys
    import types
    try:
        from antenv import axon_hooks  # noqa: F401
        return
    except ImportError:
        pass
    import antenv
    import concourse.bass_utils as _bu
    _bu.upload_artifacts = lambda tmpdir: tmpdir  # zero-egress container
    holder = {}
    mod = types.ModuleType("antenv.axon_hooks")
    mod.set_axon_ntff_profile_hook = lambda h: holder.__setitem__("h", h)
    mod.get_axon_ntff_profile_hook = lambda: holder.get("h")
    _sys.modules["antenv.axon_hooks"] = mod
    antenv.axon_hooks = mod
    from trn_agent_boot.trn_boot import _ntff_profile_via_ctypes
    mod.set_axon_ntff_profile_hook(
        _ntff_profile_via_ctypes("/opt/axon/libaxon_pjrt.so"))


def _build_graph():
    f32 = mybir.dt.float32
    bf = mybir.dt.bfloat16
    nc = bacc.Bacc("TRN2", target_bir_lowering=False, debug=False,
                   num_devices=NCORES)

    # ---- I/O ----
    xyT_d = nc.dram_tensor("xyT", [D_IN, NLOC], bf, kind="ExternalInput")
    wt0_d = nc.dram_tensor("wt0", [D_IN, H], bf, kind="ExternalInput")
    wt1_d = nc.dram_tensor("wt1", [128, KT * H], bf, kind="ExternalInput")
    wt2_d = nc.dram_tensor("wt2", [128, KT * H], bf, kind="ExternalInput")
    wt3_d = nc.dram_tensor("wt3", [128, KT * D_OUT], bf, kind="ExternalInput")
    u8 = mybir.dt.uint8
    m0_d = nc.dram_tensor("m0", [B, D_IN, H], u8, kind="ExternalInput")
    m1_d = nc.dram_tensor("m1", [B, 128, KT * H], u8, kind="ExternalInput")
    m2_d = nc.dram_tensor("m2", [B, 128, KT * H], u8, kind="ExternalInput")
    m3_d = nc.dram_tensor("m3", [B, 128, KT * D_OUT], u8, kind="ExternalInput")
    b0_d = nc.dram_tensor("b0r", [128, KT], f32, kind="ExternalInput")
    b1_d = nc.dram_tensor("b1r", [128, KT], f32, kind="ExternalInput")
    b2_d = nc.dram_tensor("b2r", [128, KT], f32, kind="ExternalInput")
    b3_d = nc.dram_tensor("b3r", [D_OUT, 1], f32, kind="ExternalInput")
    out_d = nc.dram_tensor("out", [NLOC, D_OUT], f32, kind="ExternalOutput")

    relu = mybir.ActivationFunctionType.Relu
    iden = mybir.ActivationFunctionType.Identity

    from contextlib import ExitStack
    with tile.TileContext(nc) as tc, ExitStack() as ctx:
        const = ctx.enter_context(tc.tile_pool(name="const", bufs=1))
        mwp = ctx.enter_context(tc.tile_pool(name="mw", bufs=2))
        mstg = ctx.enter_context(tc.tile_pool(name="mstg", bufs=3))
        apool = ctx.enter_context(tc.tile_pool(name="act", bufs=2))
        # 7 banks for working psums (incl. final transposes) + 1 for ps3
        pspool = ctx.enter_context(tc.tile_pool(name="ps", bufs=7, space="PSUM"))
        ps3pool = ctx.enter_context(tc.tile_pool(name="ps3", bufs=1, space="PSUM"))
        finp = ctx.enter_context(tc.tile_pool(name="fin", bufs=4))
        outtp = ctx.enter_context(tc.tile_pool(name="outT", bufs=4))

        # ---- resident constants; mask-0 critical path interleaved ----
        xyT = const.tile([D_IN, NLOC], bf, tag="xyT")
        nc.sync.dma_start(xyT[:], xyT_d[:])
        wt0 = const.tile([D_IN, H], bf, tag="wt0")
        nc.sync.dma_start(wt0[:], wt0_d[:])
        b0t = const.tile([128, KT], f32, tag="b0t")
        nc.sync.dma_start(b0t[:], b0_d[:])
        b1t = const.tile([128, KT], f32, tag="b1t")
        nc.sync.dma_start(b1t[:], b1_d[:])
        b2t = const.tile([128, KT], f32, tag="b2t")
        nc.sync.dma_start(b2t[:], b2_d[:])
        b3t = const.tile([D_OUT, 1], f32, tag="b3t")
        nc.sync.dma_start(b3t[:], b3_d[:])
        wt1 = const.tile([128, KT * H], bf, tag="wt1")
        wt2 = const.tile([128, KT * H], bf, tag="wt2")
        wt3 = const.tile([128, KT * D_OUT], bf, tag="wt3")

        # masked-weight producer for mask b: DMA the uint8 mask tile (half
        # the bytes of bf16), multiply the resident W.T slice with it on the
        # vector engine into a bf16 mw tile. m1 masks stream on gpsimd, m2 on
        # sync; bufs give ~1 mask of lookahead. For b == 0 the W.T k-slices
        # are interleaved so layer 1 starts ASAP.
        def make_mw(b):
            mw0 = mwp.tile([D_IN, H], bf, tag="mw0", bufs=2, name=f"mw0_{b}")
            mt0 = mstg.tile([D_IN, H], u8, tag="m0s", bufs=3,
                            name=f"m0s_{b}")
            nc.gpsimd.dma_start(mt0[:], m0_d[b])
            nc.vector.tensor_mul(mw0[:], wt0[:], mt0[:])
            mw1, mw2 = [], []
            for k in range(KT):
                sl = slice(k * H, (k + 1) * H)
                if b == 0:
                    nc.sync.dma_start(wt1[:, sl], wt1_d[:, sl])
                mt = mstg.tile([128, H], u8, tag="m1s", bufs=14,
                               name=f"m1s_{b}_{k}")
                nc.gpsimd.dma_start(mt[:], m1_d[b][:, sl])
                mw = mwp.tile([128, H], bf, tag=f"mw1_{k}", bufs=2,
                              name=f"mw1_{b}_{k}")
                nc.vector.tensor_mul(mw[:], wt1[:, sl], mt[:])
                mw1.append(mw)
            for k in range(KT):
                sl = slice(k * H, (k + 1) * H)
                if b == 0:
                    nc.sync.dma_start(wt2[:, sl], wt2_d[:, sl])
                mt = mstg.tile([128, H], u8, tag="m2s", bufs=14,
                               name=f"m2s_{b}_{k}")
                nc.sync.dma_start(mt[:], m2_d[b][:, sl])
                mw = mwp.tile([128, H], bf, tag=f"mw2_{k}", bufs=2,
                              name=f"mw2_{b}_{k}")
                nc.vector.tensor_mul(mw[:], wt2[:, sl], mt[:])
                mw2.append(mw)
            if b == 0:
                nc.sync.dma_start(wt3[:], wt3_d[:])
            mt3 = mstg.tile([128, KT * D_OUT], u8, tag="m3s", bufs=3,
                            name=f"m3s_{b}")
            nc.gpsimd.dma_start(mt3[:], m3_d[b])
            mw3 = mwp.tile([128, KT * D_OUT], bf, tag="mw3", bufs=2,
                           name=f"mw3_{b}")
            nc.vector.tensor_mul(mw3[:], wt3[:], mt3[:])
            return mw0, mw1, mw2, mw3

        # psum -> sbuf relu drain, alternating between the Scalar and Vector
        # engines so the drain latency (which paces psum slot recycling)
        # halves; DVE computes max(psum + bias, 0) via tensor_scalar
        add_op = mybir.AluOpType.add
        max_op = mybir.AluOpType.max

        def drain(at, ps, bt, m):
            nc.scalar.activation(at[:], ps[:], relu, bias=bt[:, m:m + 1])

        # middle layers run k-outer over m-halves of 4 so the PE consumes
        # the previous layer's activations at the rate ACT produces them
        # (m-outer stalls ~4us per mask waiting for all 8 ACTs)
        def mid_layer(b, lname, mw, a_in, bt):
            a_out = [None] * KT
            for half in range(2):
                ms = range(half * 4, half * 4 + 4)
                pss = []
                for m in ms:
                    ps = pspool.tile([128, NLOC], f32, tag="ps",
                                     name=f"ps_{lname}_{b}_{m}")
                    pss.append(ps)
                for k in range(KT):
                    for mi, m in enumerate(ms):
                        nc.tensor.matmul(pss[mi][:],
                                         mw[k][:, m * 128:(m + 1) * 128],
                                         a_in[k][:],
                                         start=(k == 0), stop=(k == KT - 1))
                for mi, m in enumerate(ms):
                    at = apool.tile([128, NLOC], bf, tag=f"{lname}_{m}",
                                    name=f"{lname}_{b}_{m}")
                    drain(at, pss[mi], bt, m)
                    a_out[m] = at
            return a_out

        # ---- main compute: all 16 masks over this core's 512 rows ----
        ps3 = ps3pool.tile([D_OUT, NLOC], f32, tag="ps3")
        for b in range(B):
            mw0, mw1, mw2, mw3 = make_mw(b)
            # layer 0: [96] -> [1024]
            a0 = []
            for m in range(KT):
                ps = pspool.tile([128, NLOC], f32, tag="ps",
                                 name=f"ps_a0_{b}_{m}")
                nc.tensor.matmul(ps[:], mw0[:, m * 128:(m + 1) * 128],
                                 xyT[:], start=True, stop=True)
                at = apool.tile([128, NLOC], bf, tag=f"a0_{m}",
                                name=f"a0_{b}_{m}")
                drain(at, ps, b0t, m)
                a0.append(at)
            a1 = mid_layer(b, "a1", mw1, a0, b1t)
            a2 = mid_layer(b, "a2", mw2, a1, b2t)
            # layer 3: [1024] -> [64]; all 16 masks accumulate in one psum
            for k in range(KT):
                lo = k * D_OUT
                nc.tensor.matmul(ps3[:], mw3[:, lo:lo + D_OUT], a2[k][:],
                                 start=(b == 0 and k == 0),
                                 stop=(b == B - 1 and k == KT - 1))

        # ---- finalize: ensemble mean + bias, transpose, store ----
        ident = const.tile([128, 128], f32, tag="ident")
        make_identity(nc, ident[:])
        for t in range(NLOC // 128):
            fin = finp.tile([D_OUT, 128], f32, tag="fin", name=f"fin_{t}")
            nc.scalar.activation(fin[:], ps3[:, t * 128:(t + 1) * 128], iden,
                                 bias=b3t[:, 0:1], scale=1.0 / B)
            pst = pspool.tile([128, D_OUT], f32, tag="ps", name=f"pst_{t}")
            nc.tensor.transpose(pst[:], fin[:], ident[:D_OUT, :D_OUT])
            ot = outtp.tile([128, D_OUT], f32, tag="ot", name=f"ot_{t}")
            nc.vector.tensor_copy(ot[:], pst[:])
            nc.sync.dma_start(out_d[t * 128:(t + 1) * 128, :], ot[:])

    nc.compile()
    return nc


def _prep_shared(W0, W1, W2, W3, b0, b1, b2, b3,
                 mask0, mask1, mask2, mask3):
    wt0 = np.ascontiguousarray(W0.T).astype(BF16)

    def kfold(wT, out_w):
        # [1024, out] -> [8, 128, out] -> [128, 8*out]
        return np.ascontiguousarray(
            wT.reshape(KT, 128, out_w).transpose(1, 0, 2).reshape(128, KT * out_w)
        ).astype(BF16)

    def mkfold(m, out_w):
        # [B, 1024, out] -> [B, 8, 128, out] -> [B, 128, 8*out]
        return np.ascontiguousarray(
            m.reshape(B, KT, 128, out_w).transpose(0, 2, 1, 3)
            .reshape(B, 128, KT * out_w)).astype(np.uint8)

    return dict(
        wt0=wt0, wt1=kfold(W1.T, H), wt2=kfold(W2.T, H),
        wt3=kfold(W3.T, D_OUT),
        m0=mask0.astype(np.uint8),
        m1=mkfold(mask1, H), m2=mkfold(mask2, H), m3=mkfold(mask3, D_OUT),
        b0r=np.ascontiguousarray(b0.reshape(KT, 128).T).astype(np.float32),
        b1r=np.ascontiguousarray(b1.reshape(KT, 128).T).astype(np.float32),
        b2r=np.ascontiguousarray(b2.reshape(KT, 128).T).astype(np.float32),
        b3r=np.ascontiguousarray(b3.reshape(D_OUT, 1)).astype(np.float32),
    )


def kernel(xy, W0, b0, W1, b1, W2, b2, W3, b3,
           mask0, mask1, mask2, mask3):
    global LAST_RESULT
    xy = np.asarray(xy, np.float32)
    args = [np.asarray(a, np.float32) for a in
            (W0, W1, W2, W3, b0, b1, b2, b3)]
    masks = [np.asarray(m, np.float32) for m in (mask0, mask1, mask2, mask3)]

    if "nc" not in _CACHE:
        _CACHE["nc"] = _build_graph()
    nc = _CACHE["nc"]

    shared = _prep_shared(*args, *masks)
    xyT = np.ascontiguousarray(xy.T).astype(BF16)   # [96, 4096]
    in_maps = []
    for core in range(NCORES):
        im = dict(shared)
        im["xyT"] = np.ascontiguousarray(
            xyT[:, core * NLOC:(core + 1) * NLOC])
        in_maps.append(im)

    if TRACE:
        _ensure_ntff_hook()
    res = run_bass_kernel_spmd(
        nc, in_maps, core_ids=list(range(NCORES)),
        trace=TRACE)
    LAST_RESULT = res
    return np.concatenate(
        [np.asarray(res.results[i]["out"], np.float32)
         for i in range(NCORES)], axis=0)
